# revision 25
# baseline (speedup 1.0000x reference)
"""Trainium2 Bass kernel for nn_CategoricalLinear (MoE-routing batched matvec).

Problem: out[b] = weight[selected_ids[b]] @ x[b]
  x: [2048, 512] f32, selected_ids: [2048] int, weight: [64, 512, 512] f32
  out: [2048, 512] f32

Strategy (category-sharded, NOT the data-parallel hint):
  - Host: stable-sort samples by category; category c's samples become a
    contiguous block. Transpose x so features lie on SBUF partitions.
  - Each of the 8 cores owns 8 categories (8 MB weight slab — the minimal
    1/8 slice of the 64 MB table) and ALL samples routed to them (~256).
  - Per category g: out_g[s, o] = sum_i x[s, i] * W_g[o, i] computed as
    4 accumulating PE matmuls: stationary = xT chunk [128(K=IN), PC(samples)],
    moving = W_g^T chunk [128(K=IN), 512(OUT)], PSUM [PC, 512].
    float32r data path -> full-rate PE (fp32 would stream at 1/4 rate).
  - Weight slab streamed per-category (1 MB DMAs) and double-buffered so the
    PE and the output path hide entirely under the weight DMA (~8 MB/core,
    the bandwidth floor for this sharding).
  - Host: unpad + inverse-permute rows back to the original sample order.

This is better than data-parallel replication: sharding the batch would make
every core read ~the whole 64 MB table (8x the aggregate HBM traffic) and
leaves ~4 samples per (core, category) matmul.
"""

import numpy as np

B, IN, OUT, C = 2048, 512, 512, 64
NCORES = 8
CPC = C // NCORES  # categories per core
KCH = IN // 128  # contraction chunks of 128


def _build_nc(
    PC,
    mm_dtype: str = "float32r",
    loop_iters: int = 0,
    unroll: int = 1,
    wbufs: int = 4,
    cats_per_dma: int = 1,
    interleave: bool = False,
    alt_rings: bool = False,
    split_first: bool = False,
):
    """Build + compile the SPMD Bass program (same NEFF runs on all 8 cores).

    PC: per-slot sample capacities (even, <= 128) — an int (uniform) or a
        sequence of CPC values. Slot g on every core holds one category
        padded to PC[g] samples.
    loop_iters: if > 0, wrap the body in a device-side For_i loop with
        `unroll` copies of the body per iteration (timing use only).
    """
    import concourse.mybir as mybir
    import concourse.tile as tile
    from concourse import bacc

    f32 = mybir.dt.float32
    mmdt = getattr(mybir.dt, mm_dtype)
    PCs = [PC] * CPC if isinstance(PC, int) else list(PC)
    assert len(PCs) == CPC
    SOFF = [0]
    for p in PCs:
        SOFF.append(SOFF[-1] + p)
    NCOL = SOFF[-1]

    nc = bacc.Bacc(
        "TRN2", target_bir_lowering=False, debug=False, num_devices=NCORES
    )
    wt = nc.dram_tensor("wt", [CPC * IN, OUT], mmdt, kind="ExternalInput").ap()
    xt = nc.dram_tensor("xt", [IN, NCOL], mmdt, kind="ExternalInput").ap()
    out = nc.dram_tensor("out", [NCOL, OUT], f32, kind="ExternalOutput").ap()

    with tile.TileContext(nc) as tc:
        with (
            tc.tile_pool(name="xp", bufs=1) as xp,
            tc.tile_pool(name="wp", bufs=wbufs) as wp,
            tc.tile_pool(name="pp", bufs=4, space="PSUM") as pp,
            tc.tile_pool(name="op", bufs=3) as op,
        ):

            def body():
                G = cats_per_dma
                if interleave:
                    # p-outer row mapping: partition p holds IN rows
                    # KCH*p + s (s=0..KCH-1). Every DMA is contiguous per
                    # partition (8 KB weight runs, one single xT DMA); the
                    # contraction over s-subsets is a row permutation the
                    # matmul accumulation doesn't care about, as long as x
                    # and W use the same mapping.
                    xt4 = xp.tile([128, KCH, NCOL], mmdt, tag="x4")
                    nc.scalar.dma_start(
                        out=xt4[:], in_=xt.rearrange("(p s) c -> p s c", p=128)
                    )
                    lhs = lambda s, g: xt4[:, s, SOFF[g] : SOFF[g] + PCs[g]]
                else:
                    xts = []
                    for k in range(KCH):
                        t = xp.tile([128, NCOL], mmdt, tag=f"x{k}")
                        # ACT ring: keep SP HWDGE free for the weight stream
                        nc.scalar.dma_start(
                            out=t[:], in_=xt[k * 128 : (k + 1) * 128, :]
                        )
                        xts.append(t)
                    lhs = lambda s, g: xts[s][:, SOFF[g] : SOFF[g] + PCs[g]]
                for gp in range(0, CPC, G):
                    # Weight block [G cats] as SBUF [128, G, KCH, OUT]. G MB/DMA.
                    wtile = wp.tile([128, G, KCH, OUT], mmdt)
                    if interleave:
                        src = wt[gp * IN : (gp + G) * IN, :].rearrange(
                            "(g p s) o -> p g s o", p=128, s=KCH
                        )
                    else:
                        src = wt[gp * IN : (gp + G) * IN, :].rearrange(
                            "(g k p) o -> p g k o", p=128, k=KCH
                        )
                    weng = (
                        nc.scalar if (alt_rings and (gp // G) % 2) else nc.sync
                    )
                    if split_first and gp == 0 and G == 1:
                        # Halve the fill latency: the first two matmuls only
                        # need k-chunks 0-1, so land them in their own DMA.
                        half = wp.tile([128, 1, KCH // 2, OUT], mmdt, tag="wh")
                        weng.dma_start(
                            out=half[:],
                            in_=wt[0 : IN // 2, :].rearrange(
                                "(g k p) o -> p g k o", p=128, k=KCH // 2
                            ),
                        )
                        weng.dma_start(
                            out=wtile[:, :, KCH // 2 :, :],
                            in_=wt[IN // 2 : IN, :].rearrange(
                                "(g k p) o -> p g k o", p=128, k=KCH // 2
                            ),
                        )
                        first_half = half
                    else:
                        weng.dma_start(out=wtile[:], in_=src)
                        first_half = None
                    for gl in range(G):
                        g = gp + gl
                        ps = pp.tile([PCs[g], OUT], f32, tag="ps")
                        for k in range(KCH):
                            if first_half is not None and k < KCH // 2:
                                rhs = first_half[:, gl, k, :]
                            else:
                                rhs = wtile[:, gl, k, :]
                            nc.tensor.matmul(
                                ps[:],
                                lhsT=lhs(k, g),
                                rhs=rhs,
                                start=(k == 0),
                                stop=(k == KCH - 1),
                            )
                        ot = op.tile([PCs[g], OUT], f32, tag="ot")
                        nc.vector.tensor_copy(out=ot[:], in_=ps[:])
                        nc.scalar.dma_start(
                            out=out[SOFF[g] : SOFF[g] + PCs[g], :], in_=ot[:]
                        )

            if loop_iters > 0:
                with tc.For_i(0, loop_iters, 1):
                    for _ in range(unroll):
                        body()
            else:
                body()
    nc.compile()
    return nc


def _prepare(x, selected_ids, weight):
    """Host-side shard prep. Returns (in_maps, meta), or (None, None) when the
    inputs don't fit the compiled layout (handled by the host fallback)."""
    x = np.ascontiguousarray(np.asarray(x, dtype=np.float32))
    ids = np.asarray(selected_ids).astype(np.int64).ravel()
    weight = np.asarray(weight, dtype=np.float32)
    if ids.size != B or ids.min() < 0 or ids.max() >= C:
        return None, None  # out-of-range ids -> host path
    counts = np.bincount(ids, minlength=C)
    mx = int(counts.max())
    if mx > 128 or weight.shape != (C, OUT, IN) or x.shape != (B, IN):
        return None, None  # pathological skew / unexpected shape -> host path
    order = np.argsort(ids, kind="stable")
    x_sorted = x[order]
    offs = np.zeros(C + 1, np.int64)
    offs[1:] = np.cumsum(counts)
    # Sorted assignment: rank categories by count (desc); slot g holds ranks
    # [8g, 8g+8) spread across the 8 cores, so slot g's padded capacity is
    # the rank-8g count (even-rounded for the fp32r fast path) instead of the
    # global max. Cuts xT/out padding bytes ~30% and balances per-core load.
    rank2cat = np.argsort(-counts, kind="stable")
    assign = rank2cat.reshape(CPC, NCORES)  # [slot, core] -> category
    PCs = [
        min(128, max(2, (int(counts[assign[g, 0]]) + 1) // 2 * 2))
        for g in range(CPC)
    ]
    SOFF = np.zeros(CPC + 1, np.int64)
    SOFF[1:] = np.cumsum(PCs)
    NCOL = int(SOFF[-1])
    wt_t = np.ascontiguousarray(weight.transpose(0, 2, 1))  # [C, IN, OUT]
    in_maps = []
    for core in range(NCORES):
        xt_k = np.zeros((IN, NCOL), np.float32)
        wlist = []
        for g in range(CPC):
            c = int(assign[g, core])
            n = int(counts[c])
            if n:
                xt_k[:, SOFF[g] : SOFF[g] + n] = x_sorted[offs[c] : offs[c + 1]].T
            wlist.append(wt_t[c])
        w_k = np.concatenate(wlist, axis=0)  # [CPC*IN, OUT]
        in_maps.append({"wt": w_k, "xt": xt_k})
    meta = dict(
        PCs=PCs, SOFF=SOFF, assign=assign, counts=counts, offs=offs, order=order
    )
    return in_maps, meta


def _gather(results, meta):
    counts, offs, order = meta["counts"], meta["offs"], meta["order"]
    assign, SOFF = meta["assign"], meta["SOFF"]
    out_sorted = np.empty((B, OUT), np.float32)
    for core in range(NCORES):
        o = results[core]["out"]
        for g in range(CPC):
            c = int(assign[g, core])
            n = int(counts[c])
            if n:
                out_sorted[offs[c] : offs[c + 1]] = o[SOFF[g] : SOFF[g] + n]
    out_full = np.empty_like(out_sorted)
    out_full[order] = out_sorted
    return out_full


_LAST = {}  # debug/test introspection: last built nc + shard maps


def kernel(x, selected_ids, weight):
    in_maps, meta = _prepare(x, selected_ids, weight)
    if in_maps is None:
        # Host fallback for inputs outside the compiled layout's assumptions.
        ids = np.asarray(selected_ids).astype(np.int64).ravel()
        w = np.asarray(weight, dtype=np.float32)
        xx = np.asarray(x, dtype=np.float32).reshape(ids.size, -1)
        outf = np.empty((ids.size, w.shape[1]), np.float32)
        for c in np.unique(ids):
            m = ids == c
            outf[m] = xx[m] @ w[c].T
        return outf
    from concourse.bass_utils import run_bass_kernel_spmd

    nc = _build_nc(meta["PCs"])
    _LAST.update(nc=nc, in_maps=in_maps, meta=meta)
    res = run_bass_kernel_spmd(nc, in_maps, core_ids=list(range(NCORES)))
    return _gather(res.results, meta)


# revision 30
# speedup vs baseline: 1.0149x; 1.0149x over previous
"""Trainium2 Bass kernel for nn_CategoricalLinear (MoE-routing batched matvec).

Problem: out[b] = weight[selected_ids[b]] @ x[b]
  x: [2048, 512] f32, selected_ids: [2048] int, weight: [64, 512, 512] f32
  out: [2048, 512] f32

Strategy (category-sharded, NOT the data-parallel hint):
  - Host: stable-sort samples by category; category c's samples become a
    contiguous block. Transpose x so features lie on SBUF partitions.
  - Each of the 8 cores owns 8 categories (8 MB weight slab — the minimal
    1/8 slice of the 64 MB table) and ALL samples routed to them (~256).
  - Per category g: out_g[s, o] = sum_i x[s, i] * W_g[o, i] computed as
    4 accumulating PE matmuls: stationary = xT chunk [128(K=IN), PC(samples)],
    moving = W_g^T chunk [128(K=IN), 512(OUT)], PSUM [PC, 512].
    float32r data path -> full-rate PE (fp32 would stream at 1/4 rate).
  - Weight slab streamed per-category (1 MB DMAs) and double-buffered so the
    PE and the output path hide entirely under the weight DMA (~8 MB/core,
    the bandwidth floor for this sharding).
  - Host: unpad + inverse-permute rows back to the original sample order.

This is better than data-parallel replication: sharding the batch would make
every core read ~the whole 64 MB table (8x the aggregate HBM traffic) and
leaves ~4 samples per (core, category) matmul.
"""

import numpy as np

B, IN, OUT, C = 2048, 512, 512, 64
NCORES = 8
CPC = C // NCORES  # categories per core
KCH = IN // 128  # contraction chunks of 128


def _build_nc(
    PC,
    mm_dtype: str = "float32r",
    loop_iters: int = 0,
    unroll: int = 1,
    wbufs: int = 4,
    cats_per_dma: int = 1,
    interleave: bool = False,
    alt_rings: bool = False,
    split_first: bool = False,
    w_engine: str = "sync",
):
    """Build + compile the SPMD Bass program (same NEFF runs on all 8 cores).

    PC: per-slot sample capacities (even, <= 128) — an int (uniform) or a
        sequence of CPC values. Slot g on every core holds one category
        padded to PC[g] samples.
    loop_iters: if > 0, wrap the body in a device-side For_i loop with
        `unroll` copies of the body per iteration (timing use only).
    """
    import concourse.mybir as mybir
    import concourse.tile as tile
    from concourse import bacc

    f32 = mybir.dt.float32
    mmdt = getattr(mybir.dt, mm_dtype)
    PCs = [PC] * CPC if isinstance(PC, int) else list(PC)
    assert len(PCs) == CPC
    SOFF = [0]
    for p in PCs:
        SOFF.append(SOFF[-1] + p)
    NCOL = SOFF[-1]

    nc = bacc.Bacc(
        "TRN2", target_bir_lowering=False, debug=False, num_devices=NCORES
    )
    wt = nc.dram_tensor("wt", [CPC * IN, OUT], mmdt, kind="ExternalInput").ap()
    xt = nc.dram_tensor("xt", [IN, NCOL], mmdt, kind="ExternalInput").ap()
    out = nc.dram_tensor("out", [NCOL, OUT], f32, kind="ExternalOutput").ap()

    with tile.TileContext(nc) as tc:
        with (
            tc.tile_pool(name="xp", bufs=1) as xp,
            tc.tile_pool(name="wp", bufs=wbufs) as wp,
            tc.tile_pool(name="pp", bufs=4, space="PSUM") as pp,
            tc.tile_pool(name="op", bufs=3) as op,
        ):

            def body():
                G = cats_per_dma
                if interleave:
                    # p-outer row mapping: partition p holds IN rows
                    # KCH*p + s (s=0..KCH-1). Every DMA is contiguous per
                    # partition (8 KB weight runs, one single xT DMA); the
                    # contraction over s-subsets is a row permutation the
                    # matmul accumulation doesn't care about, as long as x
                    # and W use the same mapping.
                    xt4 = xp.tile([128, KCH, NCOL], mmdt, tag="x4")
                    nc.scalar.dma_start(
                        out=xt4[:], in_=xt.rearrange("(p s) c -> p s c", p=128)
                    )
                    lhs = lambda s, g: xt4[:, s, SOFF[g] : SOFF[g] + PCs[g]]
                else:
                    xts = []
                    for k in range(KCH):
                        t = xp.tile([128, NCOL], mmdt, tag=f"x{k}")
                        # ACT ring: keep SP HWDGE free for the weight stream
                        nc.scalar.dma_start(
                            out=t[:], in_=xt[k * 128 : (k + 1) * 128, :]
                        )
                        xts.append(t)
                    lhs = lambda s, g: xts[s][:, SOFF[g] : SOFF[g] + PCs[g]]
                for gp in range(0, CPC, G):
                    # Weight block [G cats] as SBUF [128, G, KCH, OUT]. G MB/DMA.
                    wtile = wp.tile([128, G, KCH, OUT], mmdt)
                    if interleave:
                        src = wt[gp * IN : (gp + G) * IN, :].rearrange(
                            "(g p s) o -> p g s o", p=128, s=KCH
                        )
                    else:
                        src = wt[gp * IN : (gp + G) * IN, :].rearrange(
                            "(g k p) o -> p g k o", p=128, k=KCH
                        )
                    weng = (
                        nc.scalar
                        if (alt_rings and (gp // G) % 2)
                        else getattr(nc, w_engine)
                    )
                    if split_first and gp == 0 and G == 1:
                        # Halve the fill latency: the first two matmuls only
                        # need k-chunks 0-1, so land them in their own DMA.
                        half = wp.tile([128, 1, KCH // 2, OUT], mmdt, tag="wh")
                        weng.dma_start(
                            out=half[:],
                            in_=wt[0 : IN // 2, :].rearrange(
                                "(g k p) o -> p g k o", p=128, k=KCH // 2
                            ),
                        )
                        weng.dma_start(
                            out=wtile[:, :, KCH // 2 :, :],
                            in_=wt[IN // 2 : IN, :].rearrange(
                                "(g k p) o -> p g k o", p=128, k=KCH // 2
                            ),
                        )
                        first_half = half
                    else:
                        weng.dma_start(out=wtile[:], in_=src)
                        first_half = None
                    for gl in range(G):
                        g = gp + gl
                        ps = pp.tile([PCs[g], OUT], f32, tag="ps")
                        for k in range(KCH):
                            if first_half is not None and k < KCH // 2:
                                rhs = first_half[:, gl, k, :]
                            else:
                                rhs = wtile[:, gl, k, :]
                            nc.tensor.matmul(
                                ps[:],
                                lhsT=lhs(k, g),
                                rhs=rhs,
                                start=(k == 0),
                                stop=(k == KCH - 1),
                            )
                        ot = op.tile([PCs[g], OUT], f32, tag="ot")
                        nc.vector.tensor_copy(out=ot[:], in_=ps[:])
                        nc.scalar.dma_start(
                            out=out[SOFF[g] : SOFF[g] + PCs[g], :], in_=ot[:]
                        )

            if loop_iters > 0:
                with tc.For_i(0, loop_iters, 1):
                    for _ in range(unroll):
                        body()
            else:
                for _ in range(unroll):
                    body()
    nc.compile()
    return nc


def _prepare(x, selected_ids, weight):
    """Host-side shard prep. Returns (in_maps, meta), or (None, None) when the
    inputs don't fit the compiled layout (handled by the host fallback)."""
    x = np.ascontiguousarray(np.asarray(x, dtype=np.float32))
    ids = np.asarray(selected_ids).astype(np.int64).ravel()
    weight = np.asarray(weight, dtype=np.float32)
    if ids.size != B or ids.min() < 0 or ids.max() >= C:
        return None, None  # out-of-range ids -> host path
    counts = np.bincount(ids, minlength=C)
    mx = int(counts.max())
    if mx > 128 or weight.shape != (C, OUT, IN) or x.shape != (B, IN):
        return None, None  # pathological skew / unexpected shape -> host path
    order = np.argsort(ids, kind="stable")
    x_sorted = x[order]
    offs = np.zeros(C + 1, np.int64)
    offs[1:] = np.cumsum(counts)
    # Identity assignment, uniform capacity rounded to 16. Measured fastest on
    # HW: sorted-assignment layouts with tighter per-slot capacities moved
    # ~0.5 MB/core less but ran 0.9-1.5 us slower (shorter DMA runs / smaller
    # output blocks cost more than the saved bytes). Capacity must be EVEN or
    # the fp32r matmul fast path degrades ~2x (PC=43 measured 58.8 us).
    assign = np.arange(C).reshape(NCORES, CPC).T  # [slot, core] -> category
    PCs = [min(128, max(16, (mx + 15) // 16 * 16))] * CPC
    SOFF = np.zeros(CPC + 1, np.int64)
    SOFF[1:] = np.cumsum(PCs)
    NCOL = int(SOFF[-1])
    wt_t = np.ascontiguousarray(weight.transpose(0, 2, 1))  # [C, IN, OUT]
    in_maps = []
    for core in range(NCORES):
        xt_k = np.zeros((IN, NCOL), np.float32)
        wlist = []
        for g in range(CPC):
            c = int(assign[g, core])
            n = int(counts[c])
            if n:
                xt_k[:, SOFF[g] : SOFF[g] + n] = x_sorted[offs[c] : offs[c + 1]].T
            wlist.append(wt_t[c])
        w_k = np.concatenate(wlist, axis=0)  # [CPC*IN, OUT]
        in_maps.append({"wt": w_k, "xt": xt_k})
    meta = dict(
        PCs=PCs, SOFF=SOFF, assign=assign, counts=counts, offs=offs, order=order
    )
    return in_maps, meta


def _gather(results, meta):
    counts, offs, order = meta["counts"], meta["offs"], meta["order"]
    assign, SOFF = meta["assign"], meta["SOFF"]
    out_sorted = np.empty((B, OUT), np.float32)
    for core in range(NCORES):
        o = results[core]["out"]
        for g in range(CPC):
            c = int(assign[g, core])
            n = int(counts[c])
            if n:
                out_sorted[offs[c] : offs[c + 1]] = o[SOFF[g] : SOFF[g] + n]
    out_full = np.empty_like(out_sorted)
    out_full[order] = out_sorted
    return out_full


_LAST = {}  # debug/test introspection: last built nc + shard maps


def kernel(x, selected_ids, weight):
    in_maps, meta = _prepare(x, selected_ids, weight)
    if in_maps is None:
        # Host fallback for inputs outside the compiled layout's assumptions.
        ids = np.asarray(selected_ids).astype(np.int64).ravel()
        w = np.asarray(weight, dtype=np.float32)
        xx = np.asarray(x, dtype=np.float32).reshape(ids.size, -1)
        outf = np.empty((ids.size, w.shape[1]), np.float32)
        for c in np.unique(ids):
            m = ids == c
            outf[m] = xx[m] @ w[c].T
        return outf
    from concourse.bass_utils import run_bass_kernel_spmd

    nc = _build_nc(meta["PCs"])
    _LAST.update(nc=nc, in_maps=in_maps, meta=meta)
    res = run_bass_kernel_spmd(nc, in_maps, core_ids=list(range(NCORES)))
    return _gather(res.results, meta)


# revision 34
# speedup vs baseline: 1.0231x; 1.0081x over previous
"""Trainium2 Bass kernel for nn_CategoricalLinear (MoE-routing batched matvec).

Problem: out[b] = weight[selected_ids[b]] @ x[b]
  x: [2048, 512] f32, selected_ids: [2048] int, weight: [64, 512, 512] f32
  out: [2048, 512] f32

Strategy (category-sharded, NOT the data-parallel hint):
  - Host: stable-sort samples by category; category c's samples become a
    contiguous block. Transpose x so features lie on SBUF partitions.
  - Each of the 8 cores owns 8 categories (8 MB weight slab — the minimal
    1/8 slice of the 64 MB table) and ALL samples routed to them (~256).
  - Per category g: out_g[s, o] = sum_i x[s, i] * W_g[o, i] computed as
    4 accumulating PE matmuls: stationary = xT chunk [128(K=IN), PC(samples)],
    moving = W_g^T chunk [128(K=IN), 512(OUT)], PSUM [PC, 512].
    float32r data path -> full-rate PE (fp32 would stream at 1/4 rate).
  - Weight slab streamed per-category (1 MB DMAs) and double-buffered so the
    PE and the output path hide entirely under the weight DMA (~8 MB/core,
    the bandwidth floor for this sharding).
  - Host: unpad + inverse-permute rows back to the original sample order.

This is better than data-parallel replication: sharding the batch would make
every core read ~the whole 64 MB table (8x the aggregate HBM traffic) and
leaves ~4 samples per (core, category) matmul.
"""

import numpy as np

B, IN, OUT, C = 2048, 512, 512, 64
NCORES = 8
CPC = C // NCORES  # categories per core
KCH = IN // 128  # contraction chunks of 128


def _build_nc(
    PC,
    mm_dtype: str = "float32r",
    loop_iters: int = 0,
    unroll: int = 1,
    wbufs: int = 4,
    cats_per_dma: int = 1,
    interleave: bool = False,
    alt_rings: bool = False,
    split_first: bool = False,
    w_engine: str = "sync",
    merge_xt: bool = False,
    ppbufs: int = 4,
    opbufs: int = 3,
):
    """Build + compile the SPMD Bass program (same NEFF runs on all 8 cores).

    PC: per-slot sample capacities (even, <= 128) — an int (uniform) or a
        sequence of CPC values. Slot g on every core holds one category
        padded to PC[g] samples.
    loop_iters: if > 0, wrap the body in a device-side For_i loop with
        `unroll` copies of the body per iteration (timing use only).
    """
    import concourse.mybir as mybir
    import concourse.tile as tile
    from concourse import bacc

    f32 = mybir.dt.float32
    mmdt = getattr(mybir.dt, mm_dtype)
    PCs = [PC] * CPC if isinstance(PC, int) else list(PC)
    assert len(PCs) == CPC
    SOFF = [0]
    for p in PCs:
        SOFF.append(SOFF[-1] + p)
    NCOL = SOFF[-1]

    nc = bacc.Bacc(
        "TRN2", target_bir_lowering=False, debug=False, num_devices=NCORES
    )
    wt = nc.dram_tensor("wt", [CPC * IN, OUT], mmdt, kind="ExternalInput").ap()
    xt = nc.dram_tensor("xt", [IN, NCOL], mmdt, kind="ExternalInput").ap()
    out = nc.dram_tensor("out", [NCOL, OUT], f32, kind="ExternalOutput").ap()

    with tile.TileContext(nc) as tc:
        with (
            tc.tile_pool(name="xp", bufs=1) as xp,
            tc.tile_pool(name="wp", bufs=wbufs) as wp,
            tc.tile_pool(name="pp", bufs=ppbufs, space="PSUM") as pp,
            tc.tile_pool(name="op", bufs=opbufs) as op,
        ):

            def body():
                G = cats_per_dma
                if interleave:
                    # p-outer row mapping: partition p holds IN rows
                    # KCH*p + s (s=0..KCH-1). Every DMA is contiguous per
                    # partition (8 KB weight runs, one single xT DMA); the
                    # contraction over s-subsets is a row permutation the
                    # matmul accumulation doesn't care about, as long as x
                    # and W use the same mapping.
                    xt4 = xp.tile([128, KCH, NCOL], mmdt, tag="x4")
                    nc.scalar.dma_start(
                        out=xt4[:], in_=xt.rearrange("(p s) c -> p s c", p=128)
                    )
                    lhs = lambda s, g: xt4[:, s, SOFF[g] : SOFF[g] + PCs[g]]
                elif merge_xt:
                    # One 3-D DMA for all four k-chunks (same k-outer layout,
                    # one descriptor chain / one fixed cost on the fill path).
                    xt1 = xp.tile([128, KCH, NCOL], mmdt, tag="x1")
                    nc.scalar.dma_start(
                        out=xt1[:], in_=xt.rearrange("(k p) c -> p k c", p=128)
                    )
                    lhs = lambda s, g: xt1[:, s, SOFF[g] : SOFF[g] + PCs[g]]
                else:
                    xts = []
                    for k in range(KCH):
                        t = xp.tile([128, NCOL], mmdt, tag=f"x{k}")
                        # ACT ring: keep SP HWDGE free for the weight stream
                        nc.scalar.dma_start(
                            out=t[:], in_=xt[k * 128 : (k + 1) * 128, :]
                        )
                        xts.append(t)
                    lhs = lambda s, g: xts[s][:, SOFF[g] : SOFF[g] + PCs[g]]
                for gp in range(0, CPC, G):
                    # Weight block [G cats] as SBUF [128, G, KCH, OUT]. G MB/DMA.
                    wtile = wp.tile([128, G, KCH, OUT], mmdt)
                    if interleave:
                        src = wt[gp * IN : (gp + G) * IN, :].rearrange(
                            "(g p s) o -> p g s o", p=128, s=KCH
                        )
                    else:
                        src = wt[gp * IN : (gp + G) * IN, :].rearrange(
                            "(g k p) o -> p g k o", p=128, k=KCH
                        )
                    weng = (
                        nc.scalar
                        if (alt_rings and (gp // G) % 2)
                        else getattr(nc, w_engine)
                    )
                    if split_first and gp == 0 and G == 1:
                        # Halve the fill latency: the first two matmuls only
                        # need k-chunks 0-1, so land them in their own DMA.
                        half = wp.tile([128, 1, KCH // 2, OUT], mmdt, tag="wh")
                        weng.dma_start(
                            out=half[:],
                            in_=wt[0 : IN // 2, :].rearrange(
                                "(g k p) o -> p g k o", p=128, k=KCH // 2
                            ),
                        )
                        weng.dma_start(
                            out=wtile[:, :, KCH // 2 :, :],
                            in_=wt[IN // 2 : IN, :].rearrange(
                                "(g k p) o -> p g k o", p=128, k=KCH // 2
                            ),
                        )
                        first_half = half
                    else:
                        weng.dma_start(out=wtile[:], in_=src)
                        first_half = None
                    for gl in range(G):
                        g = gp + gl
                        ps = pp.tile([PCs[g], OUT], f32, tag="ps")
                        for k in range(KCH):
                            if first_half is not None and k < KCH // 2:
                                rhs = first_half[:, gl, k, :]
                            else:
                                rhs = wtile[:, gl, k, :]
                            nc.tensor.matmul(
                                ps[:],
                                lhsT=lhs(k, g),
                                rhs=rhs,
                                start=(k == 0),
                                stop=(k == KCH - 1),
                            )
                        ot = op.tile([PCs[g], OUT], f32, tag="ot")
                        nc.vector.tensor_copy(out=ot[:], in_=ps[:])
                        nc.scalar.dma_start(
                            out=out[SOFF[g] : SOFF[g] + PCs[g], :], in_=ot[:]
                        )

            if loop_iters > 0:
                with tc.For_i(0, loop_iters, 1):
                    for _ in range(unroll):
                        body()
            else:
                for _ in range(unroll):
                    body()
    nc.compile()
    return nc


def _prepare(x, selected_ids, weight):
    """Host-side shard prep. Returns (in_maps, meta), or (None, None) when the
    inputs don't fit the compiled layout (handled by the host fallback)."""
    x = np.ascontiguousarray(np.asarray(x, dtype=np.float32))
    ids = np.asarray(selected_ids).astype(np.int64).ravel()
    weight = np.asarray(weight, dtype=np.float32)
    if ids.size != B or ids.min() < 0 or ids.max() >= C:
        return None, None  # out-of-range ids -> host path
    counts = np.bincount(ids, minlength=C)
    mx = int(counts.max())
    if mx > 128 or weight.shape != (C, OUT, IN) or x.shape != (B, IN):
        return None, None  # pathological skew / unexpected shape -> host path
    order = np.argsort(ids, kind="stable")
    x_sorted = x[order]
    offs = np.zeros(C + 1, np.int64)
    offs[1:] = np.cumsum(counts)
    # Identity assignment, uniform capacity rounded to 16. Measured fastest on
    # HW: sorted-assignment layouts with tighter per-slot capacities moved
    # ~0.5 MB/core less but ran 0.9-1.5 us slower (shorter DMA runs / smaller
    # output blocks cost more than the saved bytes). Capacity must be EVEN or
    # the fp32r matmul fast path degrades ~2x (PC=43 measured 58.8 us).
    assign = np.arange(C).reshape(NCORES, CPC).T  # [slot, core] -> category
    PCs = [min(128, max(16, (mx + 15) // 16 * 16))] * CPC
    SOFF = np.zeros(CPC + 1, np.int64)
    SOFF[1:] = np.cumsum(PCs)
    NCOL = int(SOFF[-1])
    wt_t = np.ascontiguousarray(weight.transpose(0, 2, 1))  # [C, IN, OUT]
    in_maps = []
    for core in range(NCORES):
        xt_k = np.zeros((IN, NCOL), np.float32)
        wlist = []
        for g in range(CPC):
            c = int(assign[g, core])
            n = int(counts[c])
            if n:
                xt_k[:, SOFF[g] : SOFF[g] + n] = x_sorted[offs[c] : offs[c + 1]].T
            wlist.append(wt_t[c])
        w_k = np.concatenate(wlist, axis=0)  # [CPC*IN, OUT]
        in_maps.append({"wt": w_k, "xt": xt_k})
    meta = dict(
        PCs=PCs, SOFF=SOFF, assign=assign, counts=counts, offs=offs, order=order
    )
    return in_maps, meta


def _gather(results, meta):
    counts, offs, order = meta["counts"], meta["offs"], meta["order"]
    assign, SOFF = meta["assign"], meta["SOFF"]
    out_sorted = np.empty((B, OUT), np.float32)
    for core in range(NCORES):
        o = results[core]["out"]
        for g in range(CPC):
            c = int(assign[g, core])
            n = int(counts[c])
            if n:
                out_sorted[offs[c] : offs[c + 1]] = o[SOFF[g] : SOFF[g] + n]
    out_full = np.empty_like(out_sorted)
    out_full[order] = out_sorted
    return out_full


_LAST = {}  # debug/test introspection: last built nc + shard maps


def kernel(x, selected_ids, weight):
    in_maps, meta = _prepare(x, selected_ids, weight)
    if in_maps is None:
        # Host fallback for inputs outside the compiled layout's assumptions.
        ids = np.asarray(selected_ids).astype(np.int64).ravel()
        w = np.asarray(weight, dtype=np.float32)
        xx = np.asarray(x, dtype=np.float32).reshape(ids.size, -1)
        outf = np.empty((ids.size, w.shape[1]), np.float32)
        for c in np.unique(ids):
            m = ids == c
            outf[m] = xx[m] @ w[c].T
        return outf
    from concourse.bass_utils import run_bass_kernel_spmd

    nc = _build_nc(meta["PCs"])
    _LAST.update(nc=nc, in_maps=in_maps, meta=meta)
    res = run_bass_kernel_spmd(nc, in_maps, core_ids=list(range(NCORES)))
    return _gather(res.results, meta)
